# revision 14
# baseline (speedup 1.0000x reference)
"""MipNerf model kernel for 8 TRN2 NeuronCores (Bass/Tile), data-parallel over rays.

Layout notes (per core, 128 rays x 128 samples):
- "row" space r = n*128 + s (ray-major). MLP activations live transposed:
  [feature-partitions, rows-free], processed in 32 N-tiles of 512 rows.
- enc features are stored d-major: f = h*48 + d*16 + j (h: sin/cos half,
  d: xyz, j: freq). Host permutes w0/w5/wc rows to match, which makes every
  on-chip broadcast a plain partition-broadcast DMA.
- Rendering + resampling run in [ray-partition, sample-free] layout with
  free-dim scans (tensor_tensor_scan) for cumsum.
- MLP matmuls are bf16 (fp32 PSUM accumulate); positional encoding is exact:
  sin/exp apply the 2^j scales via per-partition ACT scale APs in fp32.
"""

import time
import numpy as np

N_RAYS = 1024
S = 128
SP1 = 129
N_CORES = 8
RAYS_PER_CORE = N_RAYS // N_CORES  # 128
ROWS = RAYS_PER_CORE * S           # 16384
NT = ROWS // 512                   # 32 N-tiles
CHUNK_NT = 4                       # N-tiles per chunk (2048 rows = 16 rays)
NCHUNK = NT // CHUNK_NT
MAX_DEG = 16
DEG_VIEW = 4
NEAR, FAR = 2.0, 6.0
RESAMPLE_PADDING = 0.01
MAGIC = 12582912.0            # 1.5 * 2^23: round-to-nearest trick
INV2PI = float(np.float32(1.0 / (2.0 * np.pi)))
C1 = 6.28125                  # 2*pi = C1 + C2 (Cody-Waite split)
C2 = float(np.float32(2.0 * np.pi - 6.28125))
RGB_PADDING = 0.001
DENSITY_BIAS = -1.0
TRUNK_DIMS = [(96, 256), (256, 256), (256, 256), (256, 256), (256, 256),
              (352, 256), (256, 256), (256, 256)]

# ---------------------------------------------------------------- permutations
def _enc_perm96():
    # new f' = d*32 + h*16 + j  maps to old f = h*48 + j*3 + d
    p = np.zeros(96, dtype=np.int64)
    for h in range(2):
        for d in range(3):
            for j in range(16):
                p[d * 32 + h * 16 + j] = h * 48 + j * 3 + d
    return p

def _vdir_perm24():
    # new f' = d*8 + h*4 + j  maps to old f = h*12 + j*3 + d
    p = np.zeros(24, dtype=np.int64)
    for h in range(2):
        for d in range(3):
            for j in range(4):
                p[d * 8 + h * 4 + j] = h * 12 + j * 3 + d
    return p

ENC_PERM = _enc_perm96()
VDIR_PERM = _vdir_perm24()

# ------------------------------------------------------------- weight packing
def _weight_blocks():
    """list of (name, ki, kr, M) in packing order; blocks are [kr<=128, M]."""
    blocks = []
    for l, (fi, fo) in enumerate(TRUNK_DIMS):
        nk = (fi + 127) // 128
        for ki in range(nk):
            kr = min(128, fi - ki * 128)
            blocks.append((f"w{l}", ki, kr, fo))
    blocks.append(("wd", 0, 128, 1))
    blocks.append(("wd", 1, 128, 1))
    blocks.append(("wb", 0, 128, 256))
    blocks.append(("wb", 1, 128, 256))
    blocks.append(("wc", 0, 128, 128))
    blocks.append(("wc", 1, 128, 128))
    blocks.append(("wc", 2, 27, 128))
    blocks.append(("wr", 0, 128, 3))
    return blocks

def _block_offsets():
    off, col = {}, 0
    for name, ki, kr, M in _weight_blocks():
        off[(name, ki)] = (col, kr, M)
        col += M
    return off, col

W_OFF, W_COLS = _block_offsets()

def _bias_cols():
    cols, c = {}, 0
    for l in range(8):
        cols[(f"b{l}", 0)] = c; c += 1
        cols[(f"b{l}", 1)] = c; c += 1
    for nm in ["bd", "bb0", "bb1", "bc", "br"]:
        cols[nm] = c; c += 1
    return cols, c

B_COLS, B_NCOLS = _bias_cols()

def _pack_host(inputs):
    """Host-side packing: layout only (f32), no value changes except row perms."""
    w = {k: np.asarray(inputs[k], dtype=np.float32) for k in inputs}
    w0p = w["w0"][ENC_PERM, :]
    w5p = w["w5"].copy()
    w5p[256:352, :] = w["w5"][256 + ENC_PERM, :]
    wcp = w["wc"].copy()
    wcp[259:283, :] = w["wc"][259 + VDIR_PERM, :]
    mats = {"w0": w0p, "w5": w5p, "wc": wcp,
            "wd": w["wd"], "wb": w["wb"], "wr": w["wr"]}
    for l in (1, 2, 3, 4, 6, 7):
        mats[f"w{l}"] = w[f"w{l}"]

    Wpack = np.zeros((128, W_COLS), dtype=np.float32)
    for name, ki, kr, M in _weight_blocks():
        col, _, _ = W_OFF[(name, ki)]
        Wpack[:kr, col:col + M] = mats[name][ki * 128: ki * 128 + kr, :]

    Bpack = np.zeros((128, B_NCOLS), dtype=np.float32)
    for l in range(8):
        b = w[f"b{l}"]
        Bpack[:, B_COLS[(f"b{l}", 0)]] = b[0:128]
        Bpack[:, B_COLS[(f"b{l}", 1)]] = b[128:256]
    Bpack[0, B_COLS["bd"]] = w["bd"][0]
    Bpack[:, B_COLS["bb0"]] = w["bb"][0:128]
    Bpack[:, B_COLS["bb1"]] = w["bb"][128:256]
    Bpack[:128, B_COLS["bc"]] = w["bc"]
    Bpack[32:35, B_COLS["br"]] = w["br"]     # rows 32..34: rgb psum lives there
    return Wpack, Bpack

def _host_consts():
    lin01 = np.linspace(0.0, 1.0, SP1).astype(np.float32)
    ures = np.linspace(0.0, 1.0 - np.finfo(np.float32).eps, SP1).astype(np.float32)
    lin01 = np.broadcast_to(lin01, (128, SP1)).copy()
    ures = np.broadcast_to(ures, (128, SP1)).copy()
    # enc order: p = d*32 + h*16 + j
    j_of_p = np.arange(96) % 16
    h_of_p = (np.arange(96) % 32) // 16
    sin_scale = (2.0 ** j_of_p).astype(np.float32).reshape(96, 1)
    sin_bias = (h_of_p * np.float32(0.5 * np.pi)).astype(np.float32).reshape(96, 1)
    exp_scale = (-0.5 * (4.0 ** j_of_p)).astype(np.float32).reshape(96, 1)
    # vdir order: p = d*8 + h*4 + j
    jv = np.arange(24) % 4
    hv = (np.arange(24) % 8) // 4
    vsin_scale = (2.0 ** jv).astype(np.float32).reshape(24, 1)
    vsin_bias = (hv * np.float32(0.5 * np.pi)).astype(np.float32).reshape(24, 1)
    return dict(lin01=lin01, ures=ures, sin_scale=sin_scale, sin_bias=sin_bias,
                exp_scale=exp_scale, vsin_scale=vsin_scale, vsin_bias=vsin_bias)

# ------------------------------------------------------------------ emission
DEBUG_TAPS = False

def build(reps=1):
    import contextlib
    import concourse.bacc as bacc
    import concourse.mybir as mybir
    import concourse.tile as tile
    from concourse.tile import add_dep_helper

    F32 = mybir.dt.float32
    BF16 = mybir.dt.bfloat16
    AF = mybir.ActivationFunctionType
    ALU = mybir.AluOpType
    AX = mybir.AxisListType

    nc = bacc.Bacc("TRN2", target_bir_lowering=False, debug=False)

    D = {}
    for nm, shp in [("origins", (128, 3)), ("directions", (128, 3)),
                    ("viewdirs", (128, 3)), ("radii", (128, 1)),
                    ("near", (128, 1)), ("far", (128, 1)),
                    ("Wpack", (128, W_COLS)), ("Bpack", (128, B_NCOLS)),
                    ("lin01", (128, SP1)), ("ures", (128, SP1)),
                    ("sin_scale", (96, 1)), ("sin_bias", (96, 1)),
                    ("exp_scale", (96, 1)), ("vsin_scale", (24, 1)),
                    ("vsin_bias", (24, 1))]:
        D[nm] = nc.dram_tensor(nm, list(shp), F32, kind="ExternalInput")
    OUT = {}
    for lvl in range(2):
        OUT[f"rgb{lvl}"] = nc.dram_tensor(f"rgb{lvl}", [128, 3], F32, kind="ExternalOutput")
        OUT[f"dist{lvl}"] = nc.dram_tensor(f"dist{lvl}", [128, 1], F32, kind="ExternalOutput")
        OUT[f"acc{lvl}"] = nc.dram_tensor(f"acc{lvl}", [128, 1], F32, kind="ExternalOutput")
    if DEBUG_TAPS:
        for nm, shp in [("dbg_tv0", (128, SP1)), ("dbg_means", (128, 3 * S)),
                        ("dbg_m96", (96, 512)), ("dbg_c96", (96, 512)),
                        ("dbg_sinx", (96, 512)), ("dbg_ev", (96, 512)),
                        ("dbg_enc", (96, 512)), ("dbg_x0", (128, 512)),
                        ("dbg_dens", (128, S)), ("dbg_rgbt", (128, 3 * S)),
                        ("dbg_cond", (27, 512)), ("dbg_tv1", (128, SP1)),
                        ("dbg_cdf", (128, SP1)), ("dbg_kt", (128, SP1)),
                        ("dbg_cdf0", (128, SP1)), ("dbg_cdf1", (128, SP1)),
                        ("dbg_wgt", (128, S))]:
            OUT[nm] = nc.dram_tensor(nm, list(shp), F32, kind="ExternalOutput")

    dma = nc.sync.dma_start
    vec = nc.vector
    act = nc.scalar

    with tile.TileContext(nc) as tc, contextlib.ExitStack() as ctx:
        const = ctx.enter_context(tc.tile_pool(name="const", bufs=1))

        # ---------------- setup: weights, biases, consts -------------------
        with tc.tile_pool(name="wstage", bufs=1) as wstage:
            w_f32 = wstage.tile([128, W_COLS], F32)
            dma(w_f32, D["Wpack"][:])
            Wbf = const.tile([128, W_COLS], BF16)
            vec.tensor_copy(Wbf, w_f32)

        Bsb = const.tile([128, B_NCOLS], F32)
        dma(Bsb, D["Bpack"][:])

        def bias_ap(key, m=None):
            c = B_COLS[key if m is None else (key, m)]
            return Bsb[:, c:c + 1]

        lin01 = const.tile([128, SP1], F32); dma(lin01, D["lin01"][:])
        ures = const.tile([128, SP1], F32); dma(ures, D["ures"][:])
        sin_scale = const.tile([96, 1], F32); dma(sin_scale, D["sin_scale"][:])
        sin_bias = const.tile([96, 1], F32); dma(sin_bias, D["sin_bias"][:])
        exp_scale = const.tile([96, 1], F32); dma(exp_scale, D["exp_scale"][:])
        vsin_scale = const.tile([24, 1], F32); dma(vsin_scale, D["vsin_scale"][:])
        vsin_bias = const.tile([24, 1], F32); dma(vsin_bias, D["vsin_bias"][:])

        dirs_t = const.tile([128, 3], F32); dma(dirs_t, D["directions"][:])
        orig_t = const.tile([128, 3], F32); dma(orig_t, D["origins"][:])
        vdirs_t = const.tile([128, 3], F32); dma(vdirs_t, D["viewdirs"][:])
        radii_t = const.tile([128, 1], F32); dma(radii_t, D["radii"][:])
        near_t = const.tile([128, 1], F32); dma(near_t, D["near"][:])
        far_t = const.tile([128, 1], F32); dma(far_t, D["far"][:])

        rad2 = const.tile([128, 1], F32)
        d_od = const.tile([128, 3], F32)
        dmag = const.tile([128, 1], F32)
        rdmag = const.tile([128, 1], F32)
        null_od = const.tile([128, 3], F32)
        dnorm = const.tile([128, 1], F32)
        fmn = const.tile([128, 1], F32)
        tmp3 = const.tile([128, 3], F32)
        vec.tensor_tensor(rad2, radii_t, radii_t, ALU.mult)
        vec.tensor_tensor(d_od, dirs_t, dirs_t, ALU.mult)
        vec.tensor_reduce(dmag, d_od, AX.X, ALU.add)
        vec.tensor_scalar_max(dmag, dmag, 1e-10)
        vec.reciprocal(rdmag, dmag)
        vec.tensor_scalar(tmp3, d_od, rdmag[:, 0:1], None, ALU.mult)
        vec.tensor_scalar(null_od, tmp3, -1.0, 1.0, ALU.mult, ALU.add)
        vec.tensor_tensor(tmp3, dirs_t, vdirs_t, ALU.mult)
        vec.tensor_reduce(dnorm, tmp3, AX.X, ALU.add)
        vec.tensor_tensor(fmn, far_t, near_t, ALU.subtract)

        # ------------- viewdir encoding -> cond [27, ROWS] bf16 ------------
        import concourse.bass as bass_mod
        vdT = const.tile([3, 128], F32)
        dma(vdT, D["viewdirs"][:].rearrange("n d -> d n"))
        # vb[p, n] = viewdirs[n, p//8]  (DRAM broadcast-transpose read)
        vb = const.tile([24, 128], F32)
        vd_ap = D["viewdirs"][:]
        for d in range(3):
            src = bass_mod.AP(tensor=vd_ap.tensor, offset=d, ap=[[0, 8], [3, 128]])
            dma(vb[d * 8:(d + 1) * 8, :], src)
        vsin = const.tile([24, 128], F32)
        vys = const.tile([24, 128], F32)
        vtk = const.tile([24, 128], F32)
        vec.tensor_scalar(vys, vb, vsin_scale[:, 0:1], vsin_bias[:, 0:1],
                          ALU.mult, ALU.add)
        vec.tensor_scalar(vtk, vys, INV2PI, MAGIC, ALU.mult, ALU.add)
        vec.tensor_scalar(vtk, vtk, -MAGIC, None, ALU.add)
        vec.scalar_tensor_tensor(vys, vtk, -C1, vys, ALU.mult, ALU.add)
        vec.scalar_tensor_tensor(vys, vtk, -C2, vys, ALU.mult, ALU.add)
        act.activation(vsin, vys, AF.Sin)
        vdT_bf = const.tile([3, 128], BF16)
        vsin_bf = const.tile([24, 128], BF16)
        vec.tensor_copy(vdT_bf, vdT)
        vec.tensor_copy(vsin_bf, vsin)
        vde_bf = const.tile([27, 128], BF16)
        dma(vde_bf[0:3, :], vdT_bf[:])
        dma(vde_bf[3:27, :], vsin_bf[:])
        # free-dim broadcast (s innermost) via DVE copy
        cond = const.tile([27, ROWS], BF16)
        vec.tensor_copy(cond.rearrange("p (n s) -> p n s", s=S),
                        vde_bf[:, :, None].broadcast_to([27, 128, S]))

        # persistent per-level tensors
        lvlbuf = ctx.enter_context(tc.tile_pool(name="lvlbuf", bufs=1))
        sinx = lvlbuf.tile([96, ROWS], BF16, tag="sinx")
        dens_t = lvlbuf.tile([128, S], F32, tag="dens_t")
        rgb_t = lvlbuf.tile([128, 3, S], F32, tag="rgb_t")
        wgt_t = lvlbuf.tile([128, S], F32, tag="rweights")

        cast_p = ctx.enter_context(tc.tile_pool(name="cast", bufs=2))
        psum = ctx.enter_context(tc.tile_pool(name="psum", bufs=2, space="PSUM"))
        psh = ctx.enter_context(tc.tile_pool(name="psh", bufs=2, space="PSUM"))

        dbgp = ctx.enter_context(tc.tile_pool(name="dbgp", bufs=1)) if DEBUG_TAPS else None

        def tap(name, ap, shape):
            if not DEBUG_TAPS or name not in OUT:
                return
            t = dbgp.tile(list(shape), F32, tag=f"t_{name}")
            vec.tensor_copy(t, ap)
            dma(OUT[name][:], t)

        def cast_rays(t_vals):
            t0 = t_vals[:, 0:S]
            t1 = t_vals[:, 1:SP1]
            mu = cast_p.tile([128, S], F32, tag="mu")
            hw2 = cast_p.tile([128, S], F32, tag="hw2")
            tdist = cast_p.tile([128, S], F32, tag="tdist")
            tmp = cast_p.tile([128, S], F32, tag="ctmp")
            tmp2 = cast_p.tile([128, S], F32, tag="ctmp2")
            rden = cast_p.tile([128, S], F32, tag="rden")
            t_mean = cast_p.tile([128, S], F32, tag="tmean")
            t_var = cast_p.tile([128, S], F32, tag="tvar")
            r_var = cast_p.tile([128, S], F32, tag="rvar")
            mu2 = cast_p.tile([128, S], F32, tag="mu2")
            vec.tensor_tensor(tdist, t1, t0, ALU.subtract)
            vec.tensor_tensor(tmp, t0, t1, ALU.add)
            vec.tensor_scalar_mul(mu, tmp, 0.5)
            vec.tensor_tensor(tmp, tdist, tdist, ALU.mult)
            vec.tensor_scalar_mul(hw2, tmp, 0.25)
            vec.tensor_tensor(mu2, mu, mu, ALU.mult)
            vec.tensor_scalar_mul(tmp, mu2, 3.0)
            vec.tensor_tensor(tmp, tmp, hw2, ALU.add)
            vec.reciprocal(rden, tmp)
            vec.tensor_tensor(tmp, mu, hw2, ALU.mult)
            vec.tensor_tensor(tmp, tmp, rden, ALU.mult)
            vec.tensor_scalar_mul(tmp, tmp, 2.0)
            vec.tensor_tensor(t_mean, mu, tmp, ALU.add)
            vec.tensor_scalar(tmp, mu2, 12.0, None, ALU.mult)
            vec.tensor_tensor(tmp, tmp, hw2, ALU.subtract)
            vec.tensor_tensor(tmp2, rden, rden, ALU.mult)
            vec.tensor_tensor(tmp, tmp, tmp2, ALU.mult)
            vec.tensor_tensor(tmp2, hw2, hw2, ALU.mult)
            vec.tensor_tensor(tmp, tmp, tmp2, ALU.mult)
            vec.tensor_scalar_mul(tmp, tmp, 4.0 / 15.0)
            vec.tensor_scalar(t_var, hw2, 1.0 / 3.0, None, ALU.mult)
            vec.tensor_tensor(t_var, t_var, tmp, ALU.subtract)
            vec.tensor_tensor(tmp, tmp2, rden, ALU.mult)
            vec.tensor_scalar_mul(tmp, tmp, 4.0 / 15.0)
            vec.tensor_scalar(r_var, mu2, 0.25, None, ALU.mult)
            vec.tensor_tensor(r_var, r_var, tmp, ALU.subtract)
            vec.tensor_scalar(tmp, hw2, 5.0 / 12.0, None, ALU.mult)
            vec.tensor_tensor(r_var, r_var, tmp, ALU.add)
            vec.tensor_scalar(r_var, r_var, rad2[:, 0:1], None, ALU.mult)
            return dict(mu=mu, tdist=tdist, t_mean=t_mean, t_var=t_var,
                        r_var=r_var)

        def level(lvl, t_vals, rep):
            cr = cast_rays(t_vals)
            means_t = cast_p.tile([128, 3, S], F32, tag="means_t")
            covs_t = cast_p.tile([128, 3, S], BF16, tag="covs_t")
            cov_f = cast_p.tile([128, 3, S], F32, tag="cov_f")
            cov_f2 = cast_p.tile([128, 3, S], F32, tag="cov_f2")
            tm_b = cr["t_mean"][:, None, :].broadcast_to([128, 3, S])
            tv_b = cr["t_var"][:, None, :].broadcast_to([128, 3, S])
            rv_b = cr["r_var"][:, None, :].broadcast_to([128, 3, S])
            dirs_b = dirs_t[:, :, None].broadcast_to([128, 3, S])
            orig_b = orig_t[:, :, None].broadcast_to([128, 3, S])
            dod_b = d_od[:, :, None].broadcast_to([128, 3, S])
            nod_b = null_od[:, :, None].broadcast_to([128, 3, S])
            vec.tensor_tensor(means_t, dirs_b, tm_b, ALU.mult)
            vec.tensor_tensor(means_t, means_t, orig_b, ALU.add)
            vec.tensor_tensor(cov_f, dod_b, tv_b, ALU.mult)
            vec.tensor_tensor(cov_f2, nod_b, rv_b, ALU.mult)
            vec.tensor_tensor(covs_t, cov_f, cov_f2, ALU.add)
            if lvl == 0 and rep == 0:
                tap("dbg_means", means_t.rearrange("p a b -> p (a b)"), (128, 3 * S))

            with tc.tile_pool(name=f"lv{lvl}_{rep}", bufs=2) as lp, \
                 tc.tile_pool(name=f"lw{lvl}_{rep}", bufs=3) as lw, \
                 tc.tile_pool(name=f"ls{lvl}_{rep}", bufs=2) as lstage:
                mean96s, cov96s = [], []
                for c in range(NCHUNK):
                    dm3 = nc.dram_tensor(f"m3b_{lvl}_{rep}_{c}", [3, 2048], F32)
                    dc3 = nc.dram_tensor(f"c3b_{lvl}_{rep}_{c}", [3, 2048], BF16)
                    for d in range(3):
                        dma(dm3[d:d + 1, :], means_t[c * 16:(c + 1) * 16, d, :])
                        dma(dc3[d:d + 1, :], covs_t[c * 16:(c + 1) * 16, d, :])
                    m96 = lp.tile([96, 2048], F32, tag="m96")
                    c96 = lp.tile([96, 2048], BF16, tag="c96")
                    for d in range(3):
                        dma(m96[d * 32:(d + 1) * 32, :],
                            dm3[d:d + 1, :].to_broadcast([32, 2048]))
                        dma(c96[d * 32:(d + 1) * 32, :],
                            dc3[d:d + 1, :].to_broadcast([32, 2048]))
                    if lvl == 0 and rep == 0 and c == 0:
                        tap("dbg_m96", m96[:, 0:512], (96, 512))
                        tap("dbg_c96", c96[:, 0:512], (96, 512))
                    mean96s.append(m96)
                    cov96s.append(c96)

                # ---------------- phase A: sines (trig table) ---------------
                last_sin = None
                for nt in range(NT):
                    c, o = nt // CHUNK_NT, (nt % CHUNK_NT) * 512
                    ys = lw.tile([96, 512], F32, tag="ys")
                    vec.tensor_scalar(ys, mean96s[c][:, o:o + 512],
                                      sin_scale[:, 0:1], sin_bias[:, 0:1],
                                      ALU.mult, ALU.add)
                    # range-reduce to [-pi, pi]: k = round(ys/2pi); r = ys - k*2pi
                    tk = lw.tile([96, 512], F32, tag="tk")
                    vec.tensor_scalar(tk, ys, INV2PI, MAGIC, ALU.mult, ALU.add)
                    vec.tensor_scalar(tk, tk, -MAGIC, None, ALU.add)
                    r1 = lw.tile([96, 512], F32, tag="r1")
                    vec.scalar_tensor_tensor(r1, tk, -C1, ys, ALU.mult, ALU.add)
                    rr = lw.tile([96, 512], F32, tag="rr")
                    vec.scalar_tensor_tensor(rr, tk, -C2, r1, ALU.mult, ALU.add)
                    last_sin = act.activation(
                        sinx[:, nt * 512:(nt + 1) * 512], rr, AF.Sin)
                if lvl == 0 and rep == 0:
                    tap("dbg_sinx", sinx[:, 0:512], (96, 512))
                    tap("dbg_cond", cond[:, 0:512], (27, 512))

                # ---------------- phase B: MLP (nat-log-exp table) ----------
                rgbd_stage = None
                for nt in range(NT):
                    c, o = nt // CHUNK_NT, (nt % CHUNK_NT) * 512
                    n0 = nt * 512
                    ev = lw.tile([96, 512], BF16, tag="ev")
                    ie = act.activation(ev, cov96s[c][:, o:o + 512], AF.Exp,
                                        scale=exp_scale[:, 0:1])
                    add_dep_helper(last_sin.ins, ie.ins, sync=False,
                                   reason="act table phase order")
                    enc = lw.tile([96, 512], BF16, tag="enc")
                    vec.tensor_tensor(enc, ev, sinx[:, n0:n0 + 512], ALU.mult)
                    if lvl == 0 and rep == 0 and nt == 0:
                        tap("dbg_ev", ev, (96, 512))
                        tap("dbg_enc", enc, (96, 512))

                    ktiles = [(enc, 96)]
                    for l in range(8):
                        fi = TRUNK_DIMS[l][0]
                        ps0 = psum.tile([128, 512], F32, tag="ps0")
                        ps1 = psum.tile([128, 512], F32, tag="ps1")
                        nk = (fi + 127) // 128
                        for ki, (xt, xr) in enumerate(ktiles):
                            col, kr, _ = W_OFF[(f"w{l}", ki)]
                            st, sp = ki == 0, ki == nk - 1
                            nc.tensor.matmul(ps0, Wbf[0:kr, col:col + 128],
                                             xt[0:xr, :], start=st, stop=sp)
                            nc.tensor.matmul(ps1, Wbf[0:kr, col + 128:col + 256],
                                             xt[0:xr, :], start=st, stop=sp)
                        x0 = lw.tile([128, 512], BF16, tag="x0")
                        x1 = lw.tile([128, 512], BF16, tag="x1")
                        act.activation(x0, ps0, AF.Relu, bias=bias_ap(f"b{l}", 0))
                        vec.tensor_scalar(x1, ps1, bias_ap(f"b{l}", 1), 0.0,
                                          ALU.add, ALU.max)
                        if lvl == 0 and rep == 0 and nt == 0 and l == 0:
                            tap("dbg_x0", x0, (128, 512))
                        ktiles = [(x0, 128), (x1, 128)]
                        if l == 4:
                            ktiles = [(x0, 128), (x1, 128), (enc, 96)]

                    x7m0, x7m1 = ktiles[0][0], ktiles[1][0]
                    hp = psh.tile([128, 512], F32, tag="psh")
                    cd0 = W_OFF[("wd", 0)][0]
                    cd1 = W_OFF[("wd", 1)][0]
                    nc.tensor.matmul(hp[0:1, :], Wbf[0:128, cd0:cd0 + 1], x7m0,
                                     start=True, stop=False)
                    nc.tensor.matmul(hp[0:1, :], Wbf[0:128, cd1:cd1 + 1], x7m1,
                                     start=False, stop=True)
                    bps0 = psum.tile([128, 512], F32, tag="ps0")
                    bps1 = psum.tile([128, 512], F32, tag="ps1")
                    cb0 = W_OFF[("wb", 0)][0]
                    cb1 = W_OFF[("wb", 1)][0]
                    nc.tensor.matmul(bps0, Wbf[0:128, cb0:cb0 + 128], x7m0, start=True, stop=False)
                    nc.tensor.matmul(bps0, Wbf[0:128, cb1:cb1 + 128], x7m1, start=False, stop=True)
                    nc.tensor.matmul(bps1, Wbf[0:128, cb0 + 128:cb0 + 256], x7m0, start=True, stop=False)
                    nc.tensor.matmul(bps1, Wbf[0:128, cb1 + 128:cb1 + 256], x7m1, start=False, stop=True)
                    bn0 = lw.tile([128, 512], BF16, tag="bn0")
                    bn1 = lw.tile([128, 512], BF16, tag="bn1")
                    act.activation(bn0, bps0, AF.Identity, bias=bias_ap("bb0"))
                    vec.tensor_scalar(bn1, bps1, bias_ap("bb1"), None, ALU.add)
                    hps = psum.tile([128, 512], F32, tag="ps0")
                    cc0 = W_OFF[("wc", 0)][0]
                    cc1 = W_OFF[("wc", 1)][0]
                    cc2 = W_OFF[("wc", 2)][0]
                    nc.tensor.matmul(hps, Wbf[0:128, cc0:cc0 + 128], bn0, start=True, stop=False)
                    nc.tensor.matmul(hps, Wbf[0:128, cc1:cc1 + 128], bn1, start=False, stop=False)
                    nc.tensor.matmul(hps, Wbf[0:27, cc2:cc2 + 128],
                                     cond[0:27, n0:n0 + 512], start=False, stop=True)
                    ht = lw.tile([128, 512], BF16, tag="h")
                    act.activation(ht, hps, AF.Relu, bias=bias_ap("bc"))
                    cr0 = W_OFF[("wr", 0)][0]
                    nc.tensor.matmul(hp[32:35, :], Wbf[0:128, cr0:cr0 + 3], ht,
                                     start=True, stop=True, tile_position=(0, 32))
                    if nt % CHUNK_NT == 0:
                        rgbd_stage = lstage.tile([35, 2048], F32, tag="rgbd")
                    vec.tensor_scalar(rgbd_stage[0:1, o:o + 512], hp[0:1, :],
                                      bias_ap("bd")[0:1, :], None, ALU.add)
                    vec.tensor_scalar(rgbd_stage[32:35, o:o + 512], hp[32:35, :],
                                      bias_ap("br")[32:35, :], None, ALU.add)
                    if nt % CHUNK_NT == CHUNK_NT - 1:
                        r0 = c * 16
                        dma(dens_t[r0:r0 + 16, :], rgbd_stage[0:1, :])
                        for ch in range(3):
                            dma(rgb_t[r0:r0 + 16, ch, :],
                                rgbd_stage[32 + ch:33 + ch, :])

            if lvl == 0 and rep == 0:
                tap("dbg_dens", dens_t, (128, S))
                tap("dbg_rgbt", rgb_t.rearrange("p a b -> p (a b)"), (128, 3 * S))
            # ---------------- rendering ------------------------------------
            with tc.tile_pool(name=f"rp{lvl}_{rep}", bufs=1) as rpool:
                xd = rpool.tile([128, S], F32)
                e1 = rpool.tile([128, S], F32)
                dens = rpool.tile([128, S], F32)
                vec.tensor_scalar(xd, dens_t, DENSITY_BIAS, 80.0, ALU.add, ALU.min)
                act.activation(e1, xd, AF.Exp)
                vec.tensor_scalar_add(e1, e1, 1.0)
                act.activation(dens, e1, AF.Ln)
                vec.tensor_tensor(dens, dens, xd, ALU.max)
                dd = rpool.tile([128, S], F32)
                vec.tensor_scalar(dd, cr["tdist"], dnorm[:, 0:1], None, ALU.mult)
                vec.tensor_tensor(dd, dd, dens, ALU.mult)
                ea = rpool.tile([128, S], F32)
                alpha = rpool.tile([128, S], F32)
                act.activation(ea, dd, AF.Exp, scale=-1.0)
                vec.tensor_scalar(alpha, ea, -1.0, 1.0, ALU.mult, ALU.add)
                cs = rpool.tile([128, S], F32)
                vec.tensor_tensor_scan(cs, dd, dd, 0.0, ALU.add, ALU.bypass)
                te = rpool.tile([128, S], F32)
                act.activation(te, cs, AF.Exp, scale=-1.0)
                vec.tensor_copy(wgt_t[:, 0:1], alpha[:, 0:1])
                vec.tensor_tensor(wgt_t[:, 1:S], alpha[:, 1:S], te[:, 0:S - 1],
                                  ALU.mult)
                er = rpool.tile([128, 3 * S], F32)
                act.activation(er, rgb_t.rearrange("p a b -> p (a b)"),
                               AF.Exp, scale=-1.0)
                vec.tensor_scalar_add(er, er, 1.0)
                sg = rpool.tile([128, 3, S], F32)
                vec.reciprocal(sg.rearrange("p a b -> p (a b)"), er)
                vec.tensor_scalar(sg.rearrange("p a b -> p (a b)"),
                                  sg.rearrange("p a b -> p (a b)"),
                                  1.0 + 2.0 * RGB_PADDING, -RGB_PADDING,
                                  ALU.mult, ALU.add)
                wrgb = rpool.tile([128, 3, S], F32)
                vec.tensor_tensor(wrgb, sg,
                                  wgt_t[:, None, :].broadcast_to([128, 3, S]),
                                  ALU.mult)
                comp = rpool.tile([128, 3], F32)
                vec.tensor_reduce(comp, wrgb, AX.X, ALU.add)
                accv = rpool.tile([128, 1], F32)
                vec.tensor_reduce(accv, wgt_t, AX.X, ALU.add)
                wt = rpool.tile([128, S], F32)
                vec.tensor_tensor(wt, wgt_t, cr["mu"], ALU.mult)
                swt = rpool.tile([128, 1], F32)
                vec.tensor_reduce(swt, wt, AX.X, ALU.add)
                racc = rpool.tile([128, 1], F32)
                vec.reciprocal(racc, accv)
                distv = rpool.tile([128, 1], F32)
                vec.tensor_tensor(distv, swt, racc, ALU.mult)
                vec.tensor_scalar(distv, distv, t_vals[:, 0:1], t_vals[:, S:SP1],
                                  ALU.max, ALU.min)
                omacc = rpool.tile([128, 1], F32)
                vec.tensor_scalar(omacc, accv, -1.0, 1.0, ALU.mult, ALU.add)
                vec.tensor_scalar(comp, comp, omacc[:, 0:1], None, ALU.add)
                dma(OUT[f"rgb{lvl}"][:], comp)
                dma(OUT[f"dist{lvl}"][:], distv)
                dma(OUT[f"acc{lvl}"][:], accv)

        def resample(rep):
            with tc.tile_pool(name=f"rs{rep}", bufs=1) as rs:
                w = wgt_t
                wmax = rs.tile([128, SP1], F32)
                vec.tensor_copy(wmax[:, 0:1], w[:, 0:1])
                vec.tensor_copy(wmax[:, S:SP1], w[:, S - 1:S])
                vec.tensor_tensor(wmax[:, 1:S], w[:, 0:S - 1], w[:, 1:S], ALU.max)
                wb_ = rs.tile([128, S], F32)
                vec.tensor_tensor(wb_, wmax[:, 0:S], wmax[:, 1:SP1], ALU.add)
                vec.tensor_scalar(wb_, wb_, 0.5, RESAMPLE_PADDING, ALU.mult, ALU.add)
                wsum = rs.tile([128, 1], F32)
                vec.tensor_reduce(wsum, wb_, AX.X, ALU.add)
                rws = rs.tile([128, 1], F32)
                vec.reciprocal(rws, wsum)
                cdf = rs.tile([128, SP1], F32)
                vec.memset(cdf[:, 0:1], 0.0)
                vec.memset(cdf[:, S:SP1], 1.0)
                vec.tensor_tensor_scan(cdf[:, 1:S], wb_[:, 0:S - 1],
                                       wb_[:, 0:S - 1], 0.0, ALU.add, ALU.bypass)
                vec.tensor_scalar(cdf[:, 1:S], cdf[:, 1:S], rws[:, 0:1], 1.0,
                                  ALU.mult, ALU.min)
                omcdf = rs.tile([128, SP1], F32)
                vec.tensor_scalar(omcdf, cdf, -1.0, 1.0, ALU.mult, ALU.add)

                kt = rs.tile([128, SP1], F32)
                cdf0 = rs.tile([128, SP1], F32)
                cdf1m = rs.tile([128, SP1], F32)
                NI = S - 1
                with tc.tile_pool(name=f"mask{rep}", bufs=1) as mp:
                    for (m0, mh) in ((0, 33), (33, 32), (65, 32), (97, 32)):
                        cdf_b = cdf[:, None, 1:S].broadcast_to([128, mh, NI])
                        omc_b = omcdf[:, None, 1:S].broadcast_to([128, mh, NI])
                        u_b = ures[:, m0:m0 + mh, None].broadcast_to([128, mh, NI])
                        mask = mp.tile([128, mh, NI], F32, tag="mask")
                        vec.tensor_tensor(mask, cdf_b, u_b, ALU.is_le)
                        vec.tensor_reduce(kt[:, m0:m0 + mh], mask, AX.X, ALU.add)
                        vec.tensor_tensor(mask, mask, cdf_b, ALU.mult)
                        vec.tensor_reduce(cdf0[:, m0:m0 + mh], mask, AX.X, ALU.max)
                        maskb = mp.tile([128, mh, NI], F32, tag="maskb")
                        vec.tensor_tensor(maskb, cdf_b, u_b, ALU.is_gt)
                        vec.tensor_tensor(maskb, maskb, omc_b, ALU.mult)
                        vec.tensor_reduce(cdf1m[:, m0:m0 + mh], maskb, AX.X,
                                          ALU.max)
                cdf1 = rs.tile([128, SP1], F32)
                vec.tensor_scalar(cdf1, cdf1m, -1.0, 1.0, ALU.mult, ALU.add)
                if rep == 0:
                    tap("dbg_cdf", cdf, (128, SP1))
                    tap("dbg_kt", kt, (128, SP1))
                    tap("dbg_cdf0", cdf0, (128, SP1))
                    tap("dbg_cdf1", cdf1, (128, SP1))
                tnum = rs.tile([128, SP1], F32)
                tden = rs.tile([128, SP1], F32)
                vec.tensor_tensor(tnum, ures, cdf0, ALU.subtract)
                vec.tensor_tensor(tden, cdf1, cdf0, ALU.subtract)
                vec.reciprocal(tden, tden)
                vec.tensor_tensor(tnum, tnum, tden, ALU.mult)
                vec.tensor_scalar(tnum, tnum, 0.0, 1.0, ALU.max, ALU.min)
                tv1 = cast_p.tile([128, SP1], F32, tag="tv1")
                vec.tensor_tensor(tv1, kt, tnum, ALU.add)
                vec.tensor_scalar(tv1, tv1, 1.0 / 32.0, 2.0, ALU.mult, ALU.add)
            return tv1

        for rep in range(reps):
            tv0 = cast_p.tile([128, SP1], F32, tag="tv0")
            vec.tensor_scalar(tv0, lin01, fmn[:, 0:1], near_t[:, 0:1],
                              ALU.mult, ALU.add)
            if rep == 0:
                tap("dbg_tv0", tv0, (128, SP1))
            level(0, tv0, rep)
            tv1 = resample(rep)
            if rep == 0:
                tap("dbg_tv1", tv1, (128, SP1))
                tap("dbg_wgt", wgt_t, (128, S))
            level(1, tv1, rep)

    nc.compile()
    return nc

# ------------------------------------------------------------------ runner
class SpmdRunner:
    """Reusable SPMD runner (axon/PJRT path), keeps the jitted callable."""

    def __init__(self, nc, n_cores=8):
        import jax
        from jax.sharding import Mesh, PartitionSpec
        from jax.experimental.shard_map import shard_map
        import concourse.mybir as mybir
        from concourse import bass2jax
        bass2jax.install_neuronx_cc_hook()
        self.jax = jax
        self.n_cores = n_cores
        partition_name = (
            nc.partition_id_tensor.name if nc.partition_id_tensor else None
        )
        in_names, out_names, out_avals, zero_outs = [], [], [], []
        for alloc in nc.m.functions[0].allocations:
            if not isinstance(alloc, mybir.MemoryLocationSet):
                continue
            name = alloc.memorylocations[0].name
            if alloc.kind == "ExternalInput":
                if name != partition_name:
                    in_names.append(name)
            elif alloc.kind == "ExternalOutput":
                out_names.append(name)
                shape = tuple(alloc.tensor_shape)
                dtype = mybir.dt.np(alloc.dtype)
                out_avals.append(jax.core.ShapedArray(shape, dtype))
                zero_outs.append(np.zeros(shape, dtype))
        self.in_names, self.out_names = in_names, out_names
        self.zero_outs, self.out_avals = zero_outs, out_avals
        n_params, n_outs = len(in_names), len(out_avals)
        in_names_all = list(in_names) + list(out_names)
        if partition_name is not None:
            in_names_all.append(partition_name)

        def _body(*args):
            operands = list(args)
            if partition_name is not None:
                operands.append(bass2jax.partition_id_tensor())
            outs = bass2jax._bass_exec_p.bind(
                *operands,
                out_avals=tuple(out_avals),
                in_names=tuple(in_names_all),
                out_names=tuple(out_names),
                lowering_input_output_aliases=(),
                sim_require_finite=True,
                sim_require_nnan=True,
                nc=nc,
            )
            return tuple(outs)

        devices = jax.devices()[:n_cores]
        mesh = Mesh(np.asarray(devices), ("core",))
        in_specs = (PartitionSpec("core"),) * (n_params + n_outs)
        out_specs = (PartitionSpec("core"),) * n_outs
        self._fn = jax.jit(
            shard_map(_body, mesh=mesh, in_specs=in_specs,
                      out_specs=out_specs, check_rep=False),
            keep_unused=True,
        )

    def prepare(self, in_maps):
        jax = self.jax
        concat_in = [
            np.concatenate(
                [np.asarray(in_maps[c][n]) for c in range(self.n_cores)], axis=0
            )
            for n in self.in_names
        ]
        concat_zeros = [
            np.zeros((self.n_cores * z.shape[0], *z.shape[1:]), z.dtype)
            for z in self.zero_outs
        ]
        return [jax.device_put(a) for a in concat_in + concat_zeros]

    def run(self, args):
        jax = self.jax
        outs = self._fn(*args)
        jax.block_until_ready(outs)
        return [
            {n: np.asarray(outs[i]).reshape(self.n_cores,
                                            *self.out_avals[i].shape)[c]
             for i, n in enumerate(self.out_names)}
            for c in range(self.n_cores)
        ]

    def time_exec(self, args, iters=10, warmup=2):
        jax = self.jax
        for _ in range(warmup):
            jax.block_until_ready(self._fn(*args))
        times = []
        for _ in range(iters):
            t0 = time.perf_counter()
            jax.block_until_ready(self._fn(*args))
            times.append(time.perf_counter() - t0)
        return min(times), float(np.median(times))

_CACHE = {}

def _get_runner(reps=1):
    if reps not in _CACHE:
        _CACHE[reps] = SpmdRunner(build(reps), N_CORES)
    return _CACHE[reps]

def _in_maps(inputs):
    consts = _host_consts()
    Wpack, Bpack = _pack_host(inputs)
    in_maps = []
    for c in range(N_CORES):
        sl = slice(c * RAYS_PER_CORE, (c + 1) * RAYS_PER_CORE)
        m = {
            "origins": np.asarray(inputs["origins"][sl], np.float32),
            "directions": np.asarray(inputs["directions"][sl], np.float32),
            "viewdirs": np.asarray(inputs["viewdirs"][sl], np.float32),
            "radii": np.asarray(inputs["radii"][sl], np.float32),
            "near": np.asarray(inputs["near"][sl], np.float32),
            "far": np.asarray(inputs["far"][sl], np.float32),
            "Wpack": Wpack, "Bpack": Bpack,
        }
        m.update(consts)
        in_maps.append(m)
    return in_maps

def kernel(**inputs):
    r = _get_runner(1)
    args = r.prepare(_in_maps(inputs))
    res = r.run(args)
    rgb0 = np.concatenate([res[c]["rgb0"] for c in range(N_CORES)], 0)
    dist0 = np.concatenate([res[c]["dist0"][:, 0] for c in range(N_CORES)], 0)
    acc0 = np.concatenate([res[c]["acc0"][:, 0] for c in range(N_CORES)], 0)
    rgb1 = np.concatenate([res[c]["rgb1"] for c in range(N_CORES)], 0)
    dist1 = np.concatenate([res[c]["dist1"][:, 0] for c in range(N_CORES)], 0)
    acc1 = np.concatenate([res[c]["acc1"][:, 0] for c in range(N_CORES)], 0)
    return (rgb0, dist0, acc0, rgb1, dist1, acc1)
